# revision 38
# baseline (speedup 1.0000x reference)
"""Multi-head attention (B=2, S=2048, D=1024, H=16, causal + rel-pos-bias + RoPE)
on 8 Trainium2 NeuronCores.

Sharding: core c handles batch c//4 and head-group c%4 (4 heads = 256 model dims).
Each core computes its heads' Q/K/V projections (column-sharded weights), RoPE,
causal attention with relative position bias, and a partial output projection
(row-sharded Wo). Host sums the 4 partials per batch and adds Wo_b.

v2: restructured for a gap-free tensor-engine stream (TRN2 p-state: the PE
only reaches 2.4 GHz after ~3us of continuous execution; any stall drops it
to 1.2 GHz). Projections run in token-half waves that chase the input DMA
stream; attention pipelines scores->exp->bias-mult->PV across quad groups
with output-projection matmuls as PE filler; elementwise work is spread
across ACT/DVE/Pool so no single engine gates the PE.
"""

import math

import numpy as np
import ml_dtypes

import concourse.bass as bass
import concourse.mybir as mybir
import concourse.tile as tile
from concourse import bacc
from concourse.bass_utils import run_bass_kernel_spmd

BF16 = ml_dtypes.bfloat16

B, S, D, H = 2, 2048, 1024, 16
DK = 64
SCALE = math.sqrt(DK)
HPC = 4          # heads per core
GDIM = HPC * DK  # 256 model dims per core
N_CORES = 8
KT = S // 128    # 16 k-tiles
QC = S // 512    # 4 q-chunks

f32 = mybir.dt.float32
f32r = mybir.dt.float32r
bf16 = mybir.dt.bfloat16

EXP = mybir.ActivationFunctionType.Exp


def _sched():
    """Attention tile schedule, shared by host bias packer and device builder.

    Yields (h, qc, kt, n, q0): head-local index, q-chunk, k-tile, the valid
    column count and starting q of the S^T tile [128 k, n q]."""
    for h in range(HPC):
        for qc in range(QC):
            for kt in range(4 * qc + 4):
                if kt // 4 == qc:  # diagonal-crossing tile
                    n = 512 - 128 * (kt % 4)
                    q0 = 128 * kt
                else:
                    n = 512
                    q0 = 512 * qc
                yield h, qc, kt, n, q0


EB_PER_HEAD = sum(128 * n for h, qc, kt, n, q0 in _sched()) // HPC
EB_TOTAL = EB_PER_HEAD * HPC

_PROGRAM = None


def _quads(qc):
    """kt quad-groups for one (h, qc) chunk: list of [(kt,n,q0)...] (4 kts)."""
    kts = list(range(4 * qc + 4))
    out = []
    for i in range(0, len(kts), 4):
        grp = []
        for kt in kts[i:i + 4]:
            if kt // 4 == qc:
                n = 512 - 128 * (kt % 4)
                q0 = 128 * kt
            else:
                n = 512
                q0 = 512 * qc
            grp.append((kt, n, q0))
        out.append(grp)
    return out


def _woffs():
    """Per-qc packed-bias offsets (within one head's EB_PER_HEAD block),
    plus per-(qc, grp-index) offsets."""
    woff_qc = {}
    acc = 0
    for qc in range(QC):
        offs = []
        for grp in _quads(qc):
            offs.append(acc)
            acc += 128 * sum(n for kt, n, q0 in grp)
        woff_qc[qc] = offs
    assert acc == EB_PER_HEAD
    return woff_qc


def _build_program():
    nc = bacc.Bacc("TRN2", target_bir_lowering=False, debug=False)

    dqT = nc.dram_tensor("qT", [8, 128, S], bf16, kind="ExternalInput").ap()
    dkT = nc.dram_tensor("kT", [8, 128, S], bf16, kind="ExternalInput").ap()
    dvT = nc.dram_tensor("vT", [8, 128, S], bf16, kind="ExternalInput").ap()
    dwq = nc.dram_tensor("wq", [128, 8 * GDIM], bf16, kind="ExternalInput").ap()
    dwk = nc.dram_tensor("wk", [128, 8 * GDIM], bf16, kind="ExternalInput").ap()
    dwv = nc.dram_tensor("wv", [128, 8 * GDIM], bf16, kind="ExternalInput").ap()
    dwo = nc.dram_tensor("wo", [128, 2 * D], bf16, kind="ExternalInput").ap()
    deb = nc.dram_tensor("eb", [EB_TOTAL], bf16, kind="ExternalInput").ap()
    dcos = nc.dram_tensor("cosT", [128, S], bf16, kind="ExternalInput").ap()
    dsin = nc.dram_tensor("sinT", [128, S], bf16, kind="ExternalInput").ap()
    dperm = nc.dram_tensor("perm", [128, 128], bf16, kind="ExternalInput").ap()
    dout = nc.dram_tensor("out", [S, D], bf16, kind="ExternalOutput").ap()

    woff_qc = _woffs()

    with tile.TileContext(nc) as tc:
        with tc.tile_pool(name="consts", bufs=1) as consts, \
             tc.tile_pool(name="xbig", bufs=1) as xbig, \
             tc.tile_pool(name="persist", bufs=1) as persist, \
             tc.tile_pool(name="ebp", bufs=1) as ebp, \
             tc.tile_pool(name="rawp", bufs=1) as rawp, \
             tc.tile_pool(name="ropep", bufs=1) as ropep, \
             tc.tile_pool(name="normp", bufs=1) as normp, \
             tc.tile_pool(name="outst", bufs=1) as outst, \
             tc.tile_pool(name="psum", bufs=1, space="PSUM") as psum:

            # ---- resident tiles ----
            wq_s = consts.tile([128, 8, GDIM], bf16)
            wk_s = consts.tile([128, 8, GDIM], bf16)
            wv_s = consts.tile([128, 8, GDIM], bf16)
            wo_s = consts.tile([128, 2, D], bf16)
            cos_s = consts.tile([128, S], bf16)
            sin_s = consts.tile([128, S], bf16)
            perm_s = consts.tile([128, 128], bf16)

            xq = xbig.tile([128, 8, 4, 512], bf16, tag="x", bufs=2, name="xq")
            xk = xbig.tile([128, 8, 4, 512], bf16, tag="x", bufs=2, name="xk")
            vfull = xbig.tile([128, 8, 4, 512], bf16, tag="x", bufs=2,
                              name="vfull")

            QT = [persist.tile([128, S], bf16, name=f"QT{m}") for m in range(2)]
            KTt = [persist.tile([128, S], bf16, name=f"KTt{m}") for m in range(2)]
            cxT = [persist.tile([128, S], bf16, name=f"cxT{m}") for m in range(2)]
            Vt = persist.tile([128, KT, HPC, DK + 1], bf16)
            nc.vector.memset(Vt[:, :, :, DK:DK + 1], 1.0)

            ones_f = consts.tile([1, DK], f32)
            nc.vector.memset(ones_f, 1.0)
            ones_r = consts.tile([1, DK], f32r)
            nc.vector.tensor_copy(out=ones_r, in_=ones_f)

            # ---- input DMA stream (sync queue; arrival ~consumption order) --
            nc.sync.dma_start(out=wq_s.rearrange("p a b -> p (a b)"), in_=dwq)
            for t in range(8):
                nc.sync.dma_start(
                    out=xq[:, t, :, :].rearrange("p a b -> p (a b)"),
                    in_=dqT[t])
            nc.sync.dma_start(out=cos_s, in_=dcos)
            nc.sync.dma_start(out=sin_s, in_=dsin)
            nc.sync.dma_start(out=perm_s, in_=dperm)
            nc.sync.dma_start(out=wk_s.rearrange("p a b -> p (a b)"), in_=dwk)
            for t in range(8):
                nc.sync.dma_start(
                    out=xk[:, t, :, :].rearrange("p a b -> p (a b)"),
                    in_=dkT[t])
            nc.sync.dma_start(out=wv_s.rearrange("p a b -> p (a b)"), in_=dwv)
            # (vfull + wo issued after the qc0 ebt DMAs below)

            # ---- RoPE, stage 1: evac one [128,512] proj psum chunk -------
            def rope_evac(pp):
                pb = ropep.tile([128, 512], bf16, tag="ppsb", bufs=4,
                                name="pb_t")
                nc.scalar.copy(out=pb, in_=pp)
                return pb

            # ---- RoPE, stage 2 (lagged one wave so the PE never waits):
            # rotate-half via a PE permutation matmul, then combine.
            def rope_fin(pb, dst, n):
                rps = psum.tile([128, 512], f32, tag="ps", bufs=2, name="rps")
                nc.tensor.matmul(rps, lhsT=perm_s, rhs=pb,
                                 start=True, stop=True)
                cc = ropep.tile([128, 512], bf16, tag="cc", bufs=2, name="cc")
                nc.gpsimd.tensor_mul(out=cc, in0=pb,
                                     in1=cos_s[:, 512 * n:512 * n + 512])
                ss = ropep.tile([128, 512], bf16, tag="ss", bufs=2, name="ss")
                nc.vector.tensor_mul(out=ss, in0=rps,
                                     in1=sin_s[:, 512 * n:512 * n + 512])
                nc.vector.tensor_add(out=dst[:, 512 * n:512 * n + 512],
                                     in0=cc, in1=ss)

            # ---- Q/K projections: token-half waves chasing the DMA ----
            pend_rope = []
            for wsrc, xsrc, dsts in ((wq_s, xq, QT), (wk_s, xk, KTt)):
                for hh in range(2):      # token half (matches DMA order)
                    for g in range(2):   # 512-token block within the half
                        pp = psum.tile([128, 2, 512], f32, tag="ps", bufs=2,
                                       name="pp")
                        for t in range(8):
                            for mi in range(2):
                                nc.tensor.matmul(
                                    pp[:, mi, :],
                                    lhsT=wsrc[:, t, 128 * mi:128 * mi + 128],
                                    rhs=xsrc[:, t, 2 * hh + g, :],
                                    start=(t == 0), stop=(t == 7))
                        nn = 2 * hh + g
                        prev_rope, pend_rope = pend_rope, []
                        for mi in range(2):
                            pend_rope.append(
                                (rope_evac(pp[:, mi, :]), dsts[mi], nn))
                        for args in prev_rope:
                            rope_fin(*args)
            for args in pend_rope:
                rope_fin(*args)

            # ---- attention quad helper: scores -> exp -> bias-mult --------
            # Emits PE work for one quad; `filler` is a callable emitting PE
            # instructions between the two score pairs (PV of the previous
            # quad / pending out-proj) to cover the exp latency.
            def emit_quad(qc, m, gi, grp, filler):
                gn = sum(n for kt, n, q0 in grp)
                woff = woff_qc[qc][gi]
                ebt = [ebp.tile([128, gn], bf16, tag=f"ebt{a}", bufs=3,
                                name=f"ebt{a}") for a in range(2)]
                for a in range(2):
                    base = (2 * m + a) * EB_PER_HEAD + woff
                    nc.sync.dma_start(
                        out=ebt[a],
                        in_=deb[base:base + 128 * gn].rearrange(
                            "(p n) -> p n", p=128))
                praw = [rawp.tile([128, gn], bf16, tag=f"pr{m}{a}", bufs=2,
                                  name=f"praw{a}") for a in range(2)]
                goff = 0
                for pi in range(0, len(grp), 2):
                    pair = grp[pi:pi + 2]
                    pn = sum(n for kt, n, q0 in pair)
                    if pi == 2 and filler is not None:
                        filler()
                    for a in range(2):
                        pss = psum.tile([128, pn], f32, tag="ps", bufs=2,
                                        name=f"pss{a}")
                        soff = 0
                        for kt, n, q0 in pair:
                            nc.tensor.matmul(
                                pss[:, soff:soff + n],
                                lhsT=KTt[m][64 * a:64 * a + DK,
                                            128 * kt:128 * kt + 128],
                                rhs=QT[m][64 * a:64 * a + DK, q0:q0 + n],
                                start=True, stop=True,
                                tile_position=(64 * a, 0))
                            soff += n
                        nc.scalar.activation(
                            out=praw[a][:, goff:goff + pn], in_=pss,
                            func=EXP)
                    goff += pn
                for a in range(2):
                    nc.vector.tensor_mul(out=praw[a], in0=praw[a], in1=ebt[a])
                return praw

            def emit_pv(qc, m, grp, praw, pcx):
                last_kt = 4 * qc + 3
                goff = 0
                for kt, n, q0 in grp:
                    co = q0 - 512 * qc
                    for a in range(2):
                        nc.tensor.matmul(
                            pcx[:, a, co:co + n],
                            lhsT=Vt[:, kt, 2 * m + a, :],
                            rhs=praw[a][:, goff:goff + n],
                            start=(kt == 0), stop=(kt == last_kt))
                    goff += n

            # norm split: _start evacuates the unnormalized context (+ the
            # softmax-denominator row DK) to SBUF with two fast ACT copies so
            # the pcx PSUM gen frees immediately; _fin runs the whole
            # reciprocal/broadcast/divide chain lazily off the SBUF copy (cxT
            # is only consumed by the out-projection one q-chunk later).
            def norm_start(qc, m, pcx):
                cxu = normp.tile([DK + 1, 2, 512], bf16, tag="cxu", bufs=2,
                                 name="cxu")
                for a in range(2):
                    nc.scalar.copy(out=cxu[:, a, :], in_=pcx[:, a, :])
                return (qc, m, cxu)

            def norm_fin(state):
                if state is None:
                    return
                qc, m, cxu = state
                for a in range(2):
                    lrow = normp.tile([1, 512], f32, tag=f"lrow{a}", bufs=1,
                                      name="lrow")
                    nc.vector.tensor_copy(out=lrow, in_=cxu[DK:DK + 1, a, :])
                    rec_f = normp.tile([1, 512], f32, tag=f"rec{a}", bufs=1,
                                       name="rec_f")
                    nc.vector.reciprocal_approx_fast(out=rec_f, in_=lrow)
                    rec = normp.tile([1, 512], f32r, tag=f"lrow{a}", bufs=1,
                                     name="rec")
                    nc.vector.tensor_copy(out=rec, in_=rec_f)
                    pb = psum.tile([DK, 512], f32, tag="pb", bufs=1, name="pb")
                    nc.tensor.matmul(pb, lhsT=ones_r, rhs=rec,
                                     start=True, stop=True)
                    nc.vector.tensor_mul(
                        out=cxT[m][64 * a:64 * a + DK,
                                   512 * qc:512 * qc + 512],
                        in0=cxu[0:DK, a, :], in1=pb)

            # ---- scores for qc=0 during the projection phase (parked) ----
            parked = {}
            for m in range(2):
                parked[m] = emit_quad(0, m, 0, _quads(0)[0], None)

            # ---- V projection: 512-token waves chasing the vT DMA --------
            for t in range(8):
                nc.sync.dma_start(
                    out=vfull[:, t, :, :].rearrange("p a b -> p (a b)"),
                    in_=dvT[t])
            nc.sync.dma_start(out=wo_s.rearrange("p a b -> p (a b)"), in_=dwo)
            for q in range(4):
                # c-major: a start=True matmul clears has_written bits for its
                # whole PSUM bank, so two live accumulation groups must never
                # share a bank ([:, c, :] chunks are half-bank each).
                pv = psum.tile([128, 4, 256], f32, tag="ps", bufs=2, name="pv")
                for c in range(4):
                    for t in range(8):
                        nc.tensor.matmul(
                            pv[:, c, :],
                            lhsT=vfull[:, t, q, 128 * c:128 * c + 128],
                            rhs=wv_s[:, t, :],
                            start=(t == 0), stop=(t == 7))
                for c in range(4):
                    nc.vector.tensor_copy(
                        out=Vt[:, 4 * q + c, :, 0:DK],
                        in_=pv[:, c, :].rearrange("p (h d) -> p h d", h=HPC))

            # ---- out-projection for one token tile (PE filler unit) ------
            def emit_po(tt):
                ost = outst.tile([128, D], bf16, tag="ost", bufs=2, name="ost")
                for e in range(2):
                    po = psum.tile([128, 512], f32, tag="po", bufs=1,
                                   name="po")
                    for m in range(2):
                        nc.tensor.matmul(
                            po,
                            lhsT=cxT[m][:, 128 * tt:128 * tt + 128],
                            rhs=wo_s[:, m, 512 * e:512 * e + 512],
                            start=(m == 0), stop=(m == 1))
                    if e == 0:
                        nc.scalar.copy(out=ost[:, 0:512], in_=po)
                    else:
                        nc.vector.tensor_copy(out=ost[:, 512:1024], in_=po)
                nc.scalar.dma_start(out=dout[128 * tt:128 * tt + 128, :],
                                    in_=ost)

            # ---- attention main loop (software-pipelined by one block) ----
            pend_po = []

            def pop_po():
                if pend_po:
                    emit_po(pend_po.pop(0))

            pending_norm = None
            pre_emitted = {}
            blocks = [(qc, m) for qc in range(QC) for m in range(2)]
            for bi, (qc, m) in enumerate(blocks):
                quads = _quads(qc)
                pcx = psum.tile([DK + 1, 2, 512], f32, tag="pcx", bufs=1,
                                name="pcx")
                nfin, pending_norm = pending_norm, None
                if qc == 0:
                    norm_fin(nfin)
                    emit_pv(0, m, quads[0], parked[m], pcx)
                else:
                    pvq = list(pre_emitted.pop((qc, m), []))
                    first = True
                    for gi in range(len(pvq), len(quads)):
                        grp = quads[gi]
                        if first:
                            fil = (lambda st=nfin: norm_fin(st))
                            first = False
                        elif pvq:
                            p = pvq.pop(0)
                            fil = (lambda p=p, px=pcx, q=qc, m_=m:
                                   emit_pv(q, m_, p[0], p[1], px))
                        else:
                            fil = pop_po
                        praw = emit_quad(qc, m, gi, grp, fil)
                        pvq.append((grp, praw))
                        pop_po()
                    if first:
                        norm_fin(nfin)
                    for p in pvq:
                        emit_pv(qc, m, p[0], p[1], pcx)
                        pop_po()
                pending_norm = norm_start(qc, m, pcx)
                if m == 1:
                    while pend_po:
                        pop_po()
                    pend_po = list(range(4 * qc, 4 * qc + 4))
                # pre-emit the next block's first score quad: cross-boundary
                # PE work so the boundary handoff never drains the PE queue
                if bi + 1 < len(blocks):
                    nqc, nm = blocks[bi + 1]
                    if nqc >= 1 and (nqc, nm) not in pre_emitted:
                        g0 = _quads(nqc)[0]
                        pre_emitted[(nqc, nm)] = [
                            (g0, emit_quad(nqc, nm, 0, g0, None))]
            norm_fin(pending_norm)
            while pend_po:
                pop_po()

    nc.compile()
    return nc


def _get_program():
    global _PROGRAM
    if _PROGRAM is None:
        _PROGRAM = _build_program()
    return _PROGRAM


def _perm_matrix():
    # P[k, m] = 1 iff k == swap(m); swap = rotate-half row permutation
    # (32-block swap within each 64-row head block).
    P = np.zeros((128, 128), dtype=BF16)
    for m in range(128):
        blk, within = m // 64, m % 64
        P[blk * 64 + (within + 32) % 64, m] = 1
    return P


def _rope_tables():
    half = DK // 2
    inv_freq = 1.0 / (10000.0 ** (np.arange(half, dtype=np.float64) / half))
    ang = np.arange(S, dtype=np.float64)[:, None] * inv_freq[None, :]  # [S, 32]
    cos = np.cos(ang).T  # [32, S]
    sin = np.sin(ang).T
    cos64 = np.concatenate([cos, cos], axis=0)            # [64, S]
    sin64 = np.concatenate([-sin, sin], axis=0)           # signed for rotate-half
    cosT = np.tile(cos64, (2, 1)).astype(BF16)            # [128, S]
    sinT = np.tile(sin64, (2, 1)).astype(BF16)
    return np.ascontiguousarray(cosT), np.ascontiguousarray(sinT)


def _pack_ebias(bias_g):
    """bias_g: [HPC, S, S] f32 (this group's heads). Returns packed 1D bf16,
    one contiguous [128, gn] row-major block per kt-quad (matching the wide
    SBUF tiles the kernel DMAs)."""
    out = np.empty(EB_TOTAL, dtype=BF16)
    off = 0
    tri = np.triu(np.ones((128, 128), dtype=np.float32))
    for h in range(HPC):
        for qc in range(QC):
            for grp in _quads(qc):
                blks = []
                for kt, n, q0 in grp:
                    blk = np.exp(
                        bias_g[h, q0:q0 + n, 128 * kt:128 * kt + 128]
                        .astype(np.float64)).T.astype(np.float32)  # [128, n]
                    if kt // 4 == qc:
                        blk[:, 0:128] *= tri
                    blks.append(blk)
                wide = np.concatenate(blks, axis=1)  # [128, gn]
                gn = wide.shape[1]
                out[off:off + 128 * gn] = wide.astype(BF16).reshape(-1)
                off += 128 * gn
    assert off == EB_TOTAL
    return out


def _prep_inputs(query, key, value, rel_pos_bias, Wq, Wk, Wv, Wo_w):
    cosT, sinT = _rope_tables()
    perm = _perm_matrix()
    xT = {}
    for nm, x in (("q", query), ("k", key), ("v", value)):
        for b in range(B):
            t = np.ascontiguousarray(x[b].T.reshape(8, 128, S)).astype(BF16)
            xT[(nm, b)] = t
    wqs, wks, wvs, wos, ebs = {}, {}, {}, {}, {}
    for g in range(4):
        sl = slice(GDIM * g, GDIM * (g + 1))
        wqs[g] = np.ascontiguousarray(
            (Wq[sl, :] / SCALE).T.reshape(8, 128, GDIM).transpose(1, 0, 2)
            .reshape(128, 8 * GDIM)).astype(BF16)
        wks[g] = np.ascontiguousarray(
            Wk[sl, :].T.reshape(8, 128, GDIM).transpose(1, 0, 2)
            .reshape(128, 8 * GDIM)).astype(BF16)
        wvs[g] = np.ascontiguousarray(
            Wv[sl, :].T.reshape(8, 128, GDIM).transpose(1, 0, 2)
            .reshape(128, 8 * GDIM)).astype(BF16)
        wos[g] = np.ascontiguousarray(
            Wo_w[:, sl].T.reshape(2, 128, D).transpose(1, 0, 2)
            .reshape(128, 2 * D)).astype(BF16)
        ebs[g] = _pack_ebias(rel_pos_bias[0, HPC * g:HPC * (g + 1)])
    in_maps = []
    for c in range(N_CORES):
        b, g = c // 4, c % 4
        in_maps.append({
            "qT": xT[("q", b)], "kT": xT[("k", b)], "vT": xT[("v", b)],
            "wq": wqs[g], "wk": wks[g], "wv": wvs[g], "wo": wos[g],
            "eb": ebs[g], "cosT": cosT, "sinT": sinT, "perm": perm,
        })
    return in_maps


def _run(query, key, value, rel_pos_bias, Wq, Wk, Wv, Wo_w, Wo_b, trace=False,
         **trace_kwargs):
    nc = _get_program()
    in_maps = _prep_inputs(query, key, value, rel_pos_bias, Wq, Wk, Wv, Wo_w)
    res = run_bass_kernel_spmd(nc, in_maps, core_ids=list(range(N_CORES)),
                               trace=trace, **trace_kwargs)
    out = np.empty((B, S, D), dtype=np.float32)
    for b in range(B):
        acc = res.results[4 * b]["out"].astype(np.float32)
        for g in range(1, 4):
            acc = acc + res.results[4 * b + g]["out"].astype(np.float32)
        out[b] = acc + Wo_b[None, :]
    return out, res


def _cpu_fallback(query, key, value, mask, rel_pos_bias, Wq, Wk, Wv, Wo_w, Wo_b):
    def rope_np(x):
        half = DK // 2
        inv_freq = 1.0 / (10000.0 ** (np.arange(half, dtype=np.float32) / half))
        ang = np.arange(S, dtype=np.float32)[:, None] * inv_freq[None, :]
        cos = np.concatenate([np.cos(ang), np.cos(ang)], axis=-1)[None, None]
        sin = np.concatenate([np.sin(ang), np.sin(ang)], axis=-1)[None, None]
        x1, x2 = x[..., :half], x[..., half:]
        rot = np.concatenate([-x2, x1], axis=-1)
        return x * cos + rot * sin

    q = np.einsum('bsd,ed->bse', query, Wq).reshape(B, S, H, DK).transpose(0, 2, 1, 3)
    k = np.einsum('bsd,ed->bse', key, Wk).reshape(B, S, H, DK).transpose(0, 2, 1, 3)
    v = np.einsum('bsd,ed->bse', value, Wv).reshape(B, S, H, DK).transpose(0, 2, 1, 3)
    q, k = rope_np(q), rope_np(k)
    sc = np.einsum('bhqd,bhkd->bhqk', q, k) / SCALE + rel_pos_bias
    sc = np.where(mask, sc, -np.inf)
    sc = sc - sc.max(axis=-1, keepdims=True)
    e = np.exp(sc)
    attn = e / e.sum(axis=-1, keepdims=True)
    ctx = np.einsum('bhqk,bhkd->bhqd', attn, v)
    ctx = ctx.transpose(0, 2, 1, 3).reshape(B, S, D)
    return (np.einsum('bsd,ed->bse', ctx, Wo_w) + Wo_b).astype(np.float32)


def kernel(query, key, value, mask, rel_pos_bias, Wq, Wk, Wv, Wo_w, Wo_b):
    query = np.asarray(query, dtype=np.float32)
    key = np.asarray(key, dtype=np.float32)
    value = np.asarray(value, dtype=np.float32)
    mask = np.asarray(mask)
    rel_pos_bias = np.asarray(rel_pos_bias, dtype=np.float32)
    Wq = np.asarray(Wq, dtype=np.float32)
    Wk = np.asarray(Wk, dtype=np.float32)
    Wv = np.asarray(Wv, dtype=np.float32)
    Wo_w = np.asarray(Wo_w, dtype=np.float32)
    Wo_b = np.asarray(Wo_b, dtype=np.float32)

    if not np.array_equal(mask.reshape(S, S),
                          np.tril(np.ones((S, S), dtype=bool))):
        return _cpu_fallback(query, key, value, mask, rel_pos_bias,
                             Wq, Wk, Wv, Wo_w, Wo_b)

    out, _ = _run(query, key, value, rel_pos_bias, Wq, Wk, Wv, Wo_w, Wo_b)
    return out
